# revision 6
# baseline (speedup 1.0000x reference)
"""BF15IntLinear on 8 TRN2 NeuronCores — v3.

Math: the reference quantizes x to "BF15" (truncate fp32 to bf16, clear
the bf16 LSB), W to truncated-bf16, then does an integer shift-align
matmul that matches an fp32-accumulated matmul of the quantized values to
~1e-5 relative.  The quantization is pure bit-twiddling, so it runs on
the host; the device sees bf16, K-major, already-sharded operands.

Per core (2 M-groups x 4 N-groups): y[256,256] = x[256,1024] @ w.T + b.
K is laid out host-side as k = 8p + j (partition p, slot j) so every DMA
lands partition-contiguous, and matmul j contracts matching k-slots of
both operands (contraction order is a free permutation).

Device program:
  - 3 input DMAs FIFO on the sync HWDGE ring, ordered so matmuls chase
    the data: C1={w j0-3 | x_m0 j0-3 | bias}, C2={w j4-7 | x_m0 j4-7},
    C3={x_m1}.
  - PE warmup: dummy matmuls into the acc0 PSUM bank while C1 streams
    (the first real matmul's start=True clears the bank, discarding
    them).  Real MATMULs keep the HAM activity window busy so the PE
    clock un-gates to 2.4 GHz before/while the real matmuls run.
  - 16 real matmuls (N=256 moving), fp32 PSUM accumulate.
  - DVE bias-add + bf16 cast per m-half; stores on scalar + sync rings.

The walrus NEFF epilogue (a per-semaphore clear loop) plus its barriers
is ~8us of fixed postamble inside the measured span — the sem-limit
patch below relocates bass's sems; the user window is what we optimize.
"""

import numpy as np
import ml_dtypes

import concourse.env as _cenv
import concourse.bass as bass
import concourse.bacc as bacc
import concourse.mybir as mybir
import concourse.bass_utils as _cbu
from concourse import tile
from concourse.bass_utils import run_bass_kernel_spmd

_SEM_LIMIT = 80


def _patched_max_sem_num() -> int:
    return _SEM_LIMIT


_cenv.get_walrus_max_sem_num = _patched_max_sem_num
bass.get_walrus_max_sem_num = _patched_max_sem_num

_orig_get_walrus_args = _cbu.get_walrus_args


def _patched_get_walrus_args(*a, **k):
    return [f"--max-sem-num={_SEM_LIMIT}", *_orig_get_walrus_args(*a, **k)]


_cbu.get_walrus_args = _patched_get_walrus_args

# Problem shape (hardcoded per contract): x [4,128,1024] f32,
# weight [1024,1024] f32, bias [1024] f32 -> out [4,128,1024] bf16.
M, K, N = 512, 1024, 1024
M_GROUPS, N_GROUPS = 2, 4
M_SH, N_SH = M // M_GROUPS, N // N_GROUPS  # 256, 256
JB = 8           # k-slots per partition: k = 8*p + j
JH = JB // 2     # j-half
MH = M_SH // 2   # m-half 128
# per-partition bf16 element counts
C1_W, C1_X, C1_B = JH * N_SH, JH * MH, N_SH      # 1024, 512, 256
C1_LEN = C1_W + C1_X + C1_B                      # 1792
C2_LEN = JH * N_SH + JH * MH                     # 1536
C3_LEN = JB * MH                                 # 1024
N_WARM = 12      # dummy N=512 matmuls bridging until C1 lands

_CACHE: dict = {}


def _build_nc():
    dt = mybir.dt
    nc = bacc.Bacc("TRN2", debug=False, target_bir_lowering=False)
    c1_d = nc.dram_tensor("c1", [128, C1_LEN], dt.bfloat16, kind="ExternalInput")
    c2_d = nc.dram_tensor("c2", [128, C2_LEN], dt.bfloat16, kind="ExternalInput")
    c3_d = nc.dram_tensor("c3", [128, C3_LEN], dt.bfloat16, kind="ExternalInput")
    y_d = nc.dram_tensor("y", [M_SH, N_SH], dt.bfloat16, kind="ExternalOutput")

    with tile.TileContext(nc) as tc:
        with (
            tc.tile_pool(name="sb", bufs=1) as pool,
            tc.tile_pool(name="acc", bufs=1, space=bass.MemorySpace.PSUM) as psacc,
        ):
            # zero operand for the warmup matmuls (gpsimd is otherwise idle)
            zt = pool.tile([128, 512], dt.bfloat16, tag="zt")
            nc.gpsimd.memset(zt[:, :], 0.0)

            # input DMAs, FIFO on the sync HWDGE ring, in consumption order
            c1 = pool.tile([128, C1_LEN], dt.bfloat16, tag="c1")
            c2 = pool.tile([128, C2_LEN], dt.bfloat16, tag="c2")
            c3 = pool.tile([128, C3_LEN], dt.bfloat16, tag="c3")
            nc.sync.dma_start(out=c1[:, :], in_=c1_d.ap())
            nc.sync.dma_start(out=c2[:, :], in_=c2_d.ap())
            nc.sync.dma_start(out=c3[:, :], in_=c3_d.ap())

            w_lo = c1[:, 0:C1_W].rearrange("p (j n) -> p j n", j=JH)
            x0_lo = c1[:, C1_W:C1_W + C1_X].rearrange("p (j m) -> p j m", j=JH)
            biasv = c1[:, C1_W + C1_X:C1_LEN]
            w_hi = c2[:, 0:C1_W].rearrange("p (j n) -> p j n", j=JH)
            x0_hi = c2[:, C1_W:C2_LEN].rearrange("p (j m) -> p j m", j=JH)
            x1v = c3[:, :].rearrange("p (j m) -> p j m", j=JB)

            acc = [
                psacc.tile([128, 512], dt.float32, tag=f"acc{mb}",
                           name=f"acc{mb}")
                for mb in range(2)
            ]

            # PE warmup: real matmuls (transpose-mode doesn't count as HAM
            # activity) with no DMA deps, writing the acc0 bank — the first
            # real matmul's start=True clears the bank, so they cost nothing
            # and need no keep-alive output.
            for _ in range(N_WARM):
                nc.tensor.matmul(acc[0][:, :], zt[:, 0:128], zt[:, :],
                                 start=True, stop=True)

            # bias upcast bf16 -> fp32 once (off critical path, during C2)
            bias_f32 = pool.tile([128, N_SH], dt.float32, tag="bias_f32")
            nc.vector.tensor_copy(bias_f32[:, :], biasv)

            # real matmuls chase the DMA chunks
            def mm(mb, j, xv, jx, wv, jw):
                nc.tensor.matmul(
                    acc[mb][:, 0:N_SH], xv[:, jx, :], wv[:, jw, :],
                    start=(j == 0), stop=(j == JB - 1),
                )

            for j in range(JH):
                mm(0, j, x0_lo, j, w_lo, j)
            for j in range(JH, JB):
                mm(0, j, x0_hi, j - JH, w_hi, j - JH)
            for j in range(JH):
                mm(1, j, x1v, j, w_lo, j)
            for j in range(JH, JB):
                mm(1, j, x1v, j, w_hi, j - JH)

            # epilogue + store, per m-half on separate trigger queues
            ysb = pool.tile([128, 2, N_SH], dt.bfloat16, tag="ysb")
            y_dst = y_d.ap().rearrange("(mb p) n -> p mb n", p=128)
            for mb in range(2):
                nc.vector.tensor_tensor(
                    out=ysb[:, mb, :], in0=acc[mb][:, 0:N_SH],
                    in1=bias_f32[:, :], op=mybir.AluOpType.add,
                )
                eng = nc.scalar if mb == 0 else nc.sync
                eng.dma_start(out=y_dst[:, mb, :], in_=ysb[:, mb, :])

    nc.compile()
    return nc


def get_nc():
    if "nc" not in _CACHE:
        _CACHE["nc"] = _build_nc()
    return _CACHE["nc"]


def _trunc_bf16_u16(a: np.ndarray, clear_lsb: bool) -> np.ndarray:
    """fp32 -> truncated-bf16 bit pattern (toward zero); BF15 clears LSB."""
    u = (np.ascontiguousarray(a, dtype=np.float32).view(np.uint32) >> 16
         ).astype(np.uint16)
    if clear_lsb:
        u &= np.uint16(0xFFFE)
    return u


def make_in_maps(x: np.ndarray, weight: np.ndarray, bias: np.ndarray):
    xq = _trunc_bf16_u16(np.asarray(x).reshape(M, K), clear_lsb=True)
    wq = _trunc_bf16_u16(np.asarray(weight), clear_lsb=False)
    bq = _trunc_bf16_u16(np.asarray(bias), clear_lsb=False)

    in_maps = []
    for core in range(M_GROUPS * N_GROUPS):
        mi, ni = divmod(core, N_GROUPS)
        xT = np.ascontiguousarray(xq[mi * M_SH:(mi + 1) * M_SH, :].T)  # [K, 256]
        wT = np.ascontiguousarray(wq[ni * N_SH:(ni + 1) * N_SH, :].T)  # [K, 256]
        bs = bq[ni * N_SH:(ni + 1) * N_SH]                             # [256]
        wk = wT.reshape(128, JB, N_SH)       # (p, j, n)
        x0k = np.ascontiguousarray(xT[:, 0:MH]).reshape(128, JB, MH)
        x1k = np.ascontiguousarray(xT[:, MH:M_SH]).reshape(128, JB, MH)
        c1 = np.empty((128, C1_LEN), np.uint16)
        c1[:, 0:C1_W] = wk[:, 0:JH].reshape(128, C1_W)
        c1[:, C1_W:C1_W + C1_X] = x0k[:, 0:JH].reshape(128, C1_X)
        c1[:, C1_W + C1_X:] = np.broadcast_to(bs, (128, N_SH))
        c2 = np.empty((128, C2_LEN), np.uint16)
        c2[:, 0:C1_W] = wk[:, JH:JB].reshape(128, C1_W)
        c2[:, C1_W:] = x0k[:, JH:JB].reshape(128, C1_X)
        c3 = np.ascontiguousarray(x1k.reshape(128, C3_LEN))
        in_maps.append({
            "c1": c1.view(ml_dtypes.bfloat16),
            "c2": c2.view(ml_dtypes.bfloat16),
            "c3": c3.view(ml_dtypes.bfloat16),
        })
    return in_maps


def assemble(results) -> np.ndarray:
    y2d = np.empty((M, N), dtype=ml_dtypes.bfloat16)
    for c in range(M_GROUPS * N_GROUPS):
        mi, ni = divmod(c, N_GROUPS)
        y2d[mi * M_SH:(mi + 1) * M_SH, ni * N_SH:(ni + 1) * N_SH] = results[c]["y"]
    return y2d.reshape(4, 128, N)


def kernel(x: np.ndarray, weight: np.ndarray, bias: np.ndarray) -> np.ndarray:
    nc = get_nc()
    in_maps = make_in_maps(x, weight, bias)
    res = run_bass_kernel_spmd(nc, in_maps, core_ids=list(range(8)))
    return assemble(res.results)
